# revision 6
# baseline (speedup 1.0000x reference)
"""Trainium2 Bass kernel for nn_MultiHeadAttention_28819230556860 (v2).

SimA (softmax-free) multi-head attention in chunked linear form:
    out_blk = tril(q k^T) v_blk + S^T q_blk,   S += k_blk v_blk^T

v2 restructuring vs v1 (169us):
  * k is projected ONCE (head-major) and transposed on the PE array to get
    the token-major copy needed for the state update (saves a full C x C
    projection per core).
  * Inputs are host-packed into SBUF layout so each tensor is a single
    dma_start (the sync engine serializes dma dispatches at ~600ns each;
    50 dispatches starved phase A of x data).
  * All phases are emitted so the tensor engine never idles: v-projection
    interleaved with q/k stage-1, attention processes the 3 head-pairs
    round-robin with next-block scores pipelined inside the current block,
    output projection as a dense tail.  Continuous PE occupancy keeps the
    engine at the 2.4 GHz p-state instead of 1.2 GHz.
  * PSUM->SBUF copies spread across Vector/Scalar so neither gates the PE.

PSUM zero-region rule: start=True marks the touched partitions' ENTIRE 2KB
bank row pending-zero.  When several accumulation groups share a bank, only
the first matmul in the bank may use start=True; later groups' first writes
find pending-zero bytes and reset lazily.

Sharding: 8 cores = 4 batches x 2 head-groups (6 heads each).  Each core
computes a partial (T, C) output (row-sharded Wp); the host sums core pairs.
"""

import os
import sys

sys.path.insert(0, "/opt/trn_rl_repo")

from contextlib import ExitStack

import ml_dtypes
import numpy as np

import concourse.bass as bass
import concourse.tile as tile
from concourse import bacc, mybir

# ---------------------------------------------------------------------------
# Problem constants (hardcoded from the reference nn.Module).
# ---------------------------------------------------------------------------
B, T, C = 4, 2048, 768
H, D = 12, 64
P = 128
KC = C // P          # 6 contraction tiles over the embedding dim
NP = 3               # head-pairs per core (6 heads, 2 per 128 partitions)
NPP = NP * P         # 384
TW = 512             # t-window for 512-wide psum tiles
NTW = T // TW        # 4
NTB = T // P         # 16 token blocks
XW = KC * TW         # packed x columns per t-window (3072)
SCALE = float(C) ** -0.5
EPS = 1e-12

F32 = mybir.dt.float32


def build_nc(dt=F32):
    nc = bacc.Bacc("TRN2", target_bir_lowering=False, debug=False,
                   enable_asserts=False)

    # All inputs host-packed to [128, .] SBUF layout; single DMA each.
    xt_d = nc.dram_tensor("xt", [P, KC * T], dt, kind="ExternalInput").ap()
    wv_d = nc.dram_tensor("wv", [P, KC * NPP], dt, kind="ExternalInput").ap()
    wq_d = nc.dram_tensor("wq", [P, KC * NPP], dt, kind="ExternalInput").ap()
    wk_d = nc.dram_tensor("wk", [P, KC * NPP], dt, kind="ExternalInput").ap()
    wp_d = nc.dram_tensor("wp", [P, NP * C], dt, kind="ExternalInput").ap()
    mask_d = nc.dram_tensor("mask3", [P, NPP], dt, kind="ExternalInput").ap()
    id_d = nc.dram_tensor("ident", [P, P], dt, kind="ExternalInput").ap()
    bd_d = nc.dram_tensor("bdmask", [P, P], dt, kind="ExternalInput").ap()
    out = nc.dram_tensor("out", [T, C], dt, kind="ExternalOutput").ap()

    with tile.TileContext(nc) as tc:
        with ExitStack() as ctx:
            _body(ctx, tc, dt, xt_d, wv_d, wq_d, wk_d, wp_d, mask_d, id_d,
                  bd_d, out)
    nc.compile()
    return nc


def _body(ctx, tc, dt, xt_d, wv_d, wq_d, wk_d, wp_d, mask_d, id_d, bd_d,
          out):
    nc = tc.nc
    OP = mybir.AluOpType
    AF = mybir.ActivationFunctionType

    consts = ctx.enter_context(tc.tile_pool(name="consts", bufs=1))

    # ---- SBUF const tiles --------------------------------------------------
    xt = consts.tile([P, KC * T], dt, name="xt", tag="xt")
    wv = consts.tile([P, KC * NPP], dt, name="wv", tag="wv")
    wq = consts.tile([P, KC * NPP], dt, name="wq", tag="wq")
    wk = consts.tile([P, KC * NPP], dt, name="wk", tag="wk")
    wp = consts.tile([P, NP * C], dt, name="wp", tag="wp")
    mask_sb = consts.tile([P, NPP], dt, name="mask_sb", tag="mask_sb")
    id_sb = consts.tile([P, P], dt, name="id_sb", tag="id_sb")
    bd_sb = consts.tile([P, P], dt, name="bd_sb", tag="bd_sb")

    # packed x slices: window tw, contraction tile kc
    def xw(tw, kc, j0, n):
        c0 = tw * XW + kc * TW + j0
        return xt[:, c0:c0 + n]

    # ---- DMA schedule (few large transfers, prioritized) -------------------
    nc.sync.dma_start(wv[:], wv_d[:])
    nc.sync.dma_start(xt[:, 0:XW], xt_d[:, 0:XW])
    nc.sync.dma_start(wq[:], wq_d[:])
    nc.sync.dma_start(wk[:], wk_d[:])
    for tw in range(1, NTW):
        nc.sync.dma_start(xt[:, tw * XW:(tw + 1) * XW],
                          xt_d[:, tw * XW:(tw + 1) * XW])
    nc.sync.dma_start(mask_sb[:], mask_d[:])
    nc.sync.dma_start(id_sb[:], id_d[:])
    nc.sync.dma_start(bd_sb[:], bd_d[:])
    nc.sync.dma_start(wp[:], wp_d[:])

    # ---- SBUF pools --------------------------------------------------------
    vpool = ctx.enter_context(tc.tile_pool(name="vpool", bufs=1))
    qkpool = ctx.enter_context(tc.tile_pool(name="qkpool", bufs=1))
    spool = ctx.enter_context(tc.tile_pool(name="spool", bufs=2))
    opool = ctx.enter_context(tc.tile_pool(name="opool", bufs=1))
    weipool = ctx.enter_context(tc.tile_pool(name="weipool", bufs=4))
    sspool = ctx.enter_context(tc.tile_pool(name="sspool", bufs=4))
    prpool = ctx.enter_context(tc.tile_pool(name="prpool", bufs=4))

    v_t = [vpool.tile([P, NPP], dt, name=f"v{tb}", tag=f"v{tb}")
           for tb in range(NTB)]
    kn_t = [vpool.tile([P, NPP], dt, name=f"kn{tb}", tag=f"kn{tb}")
            for tb in range(NTB)]
    qT = [qkpool.tile([P, T], dt, name=f"qT{p}", tag=f"qT{p}")
          for p in range(NP)]
    kT = [qkpool.tile([P, T], dt, name=f"kT{p}", tag=f"kT{p}")
          for p in range(NP)]
    qTz = [qkpool.tile([P, 2 * T], dt, name=f"qTz{p}", tag=f"qTz{p}")
           for p in range(NP)]
    for p in range(NP):
        nc.gpsimd.memset(qTz[p][:], 0.0)
    outT = [opool.tile([P, T], dt, name=f"outT{p}", tag=f"outT{p}")
            for p in range(NP)]
    sqp = [spool.tile([P, NTW], F32, name=f"sqp{p}", tag=f"sqp{p}", bufs=1)
           for p in range(NP)]
    skp = [spool.tile([P, NTW], F32, name=f"skp{p}", tag=f"skp{p}", bufs=1)
           for p in range(NP)]
    fac = [spool.tile([P, 1], F32, name=f"fac{p}", tag=f"fac{p}", bufs=1)
           for p in range(NP)]

    # =======================================================================
    # Phase A: projections (v token-major; q,k head-major), k transposes,
    # norm factors.  Emitted so the PE stream has no dependency stalls.
    # =======================================================================
    phA = ExitStack()
    ps_mm = phA.enter_context(tc.tile_pool(name="ps_mm", bufs=3, space="PSUM"))
    ps_tr = phA.enter_context(tc.tile_pool(name="ps_tr", bufs=4, space="PSUM"))

    def emit_vproj(tb, eng):
        tw, j0 = tb // 4, (tb % 4) * P
        pv = ps_mm.tile([P, TW], F32, name="pv", tag="mm")[:, :NPP]
        for kc in range(KC):
            nc.tensor.matmul(pv, lhsT=xw(tw, kc, j0, P),
                             rhs=wv[:, kc * NPP:(kc + 1) * NPP],
                             start=(kc == 0), stop=(kc == KC - 1))
        (nc.vector.tensor_copy if eng == 0 else nc.scalar.copy)(
            out=v_t[tb][:], in_=pv)

    def emit_stage1(p, tw, which):
        wsl = slice(tw * TW, (tw + 1) * TW)
        w_t = wq if which == "q" else wk
        dst = qT[p] if which == "q" else kT[p]
        acc = sqp[p] if which == "q" else skp[p]
        pq = ps_mm.tile([P, TW], F32, name="pq", tag="mm")
        for kc in range(KC):
            nc.tensor.matmul(
                pq, lhsT=w_t[:, kc * NPP + p * P:kc * NPP + (p + 1) * P],
                rhs=xw(tw, kc, 0, TW),
                start=(kc == 0), stop=(kc == KC - 1))
        nc.vector.tensor_copy(out=dst[:, wsl], in_=pq)
        scr = spool.tile([P, TW], F32, name="scr", tag="scr", bufs=2)
        nc.scalar.activation(out=scr[:], in_=pq, func=AF.Square,
                             accum_out=acc[:, tw:tw + 1])

    pending_tr = []

    def drain_tr(n):
        # kT[p][:, block] -> token-major kn tiles, via PE transpose.  Emitted
        # a couple at a time between long matmul groups so the PE never
        # queues behind the psum->sbuf copies.
        for _ in range(min(n, len(pending_tr))):
            p, tb = pending_tr.pop(0)
            tsl = slice(tb * P, (tb + 1) * P)
            pt = ps_tr.tile([P, P], dt, name="pt", tag="tr")
            nc.tensor.transpose(pt, kT[p][:, tsl], id_sb[:])
            ((nc.vector.tensor_copy if (p + tb) % 2 else nc.scalar.copy)
             (out=kn_t[tb][:, p * P:(p + 1) * P], in_=pt))

    def emit_norm_scale(p):
        # fac[d] = SCALE / max(sqrt(||q_d||^2 * ||k_d||^2), eps); fold into q
        nq = spool.tile([P, 1], F32, name="nq", tag="nq")
        nk = spool.tile([P, 1], F32, name="nk", tag="nk")
        s4 = spool.tile([P, NTW], F32, name="s4", tag="s4")
        nc.scalar.activation(out=s4[:], in_=sqp[p][:], func=AF.Copy,
                             accum_out=nq[:])
        s5 = spool.tile([P, NTW], F32, name="s5", tag="s4")
        nc.scalar.activation(out=s5[:], in_=skp[p][:], func=AF.Copy,
                             accum_out=nk[:])
        nc.vector.tensor_mul(fac[p][:], nq[:], nk[:])
        nc.scalar.sqrt(fac[p][:], fac[p][:])
        nc.vector.tensor_scalar_max(fac[p][:], fac[p][:], EPS)
        nc.vector.reciprocal(fac[p][:], fac[p][:])
        nc.vector.tensor_scalar_mul(fac[p][:], fac[p][:], SCALE)
        for tw in range(NTW):
            wsl = slice(tw * TW, (tw + 1) * TW)
            if tw % 2:
                nc.scalar.mul(out=qT[p][:, wsl], in_=qT[p][:, wsl],
                              mul=fac[p][:])
            else:
                nc.vector.tensor_scalar_mul(qT[p][:, wsl], qT[p][:, wsl],
                                            fac[p][:])

    # --- phase A emission order (PE stream) ---
    for tw in range(NTW):
        emit_vproj(2 * tw, 0)
        drain_tr(2)
        emit_vproj(2 * tw + 1, 1)
        drain_tr(2)
        for p in range(NP):
            emit_stage1(p, tw, "q")
            drain_tr(2)
        for p in range(NP):
            emit_stage1(p, tw, "k")
            drain_tr(2)
            pending_tr.extend((p, tw * 4 + i) for i in range(4))
    def emit_qtz(p, tw, eng):
        # zero-interleaved q for fused two-head scores (zeros laid down by
        # the one-time memset above):
        # qTz[0:64, tb*256+0:128] = qA(tb), qTz[64:128, tb*256+128:256] = qB(tb)
        for h in range(2):
            hsl = slice(h * D, (h + 1) * D)
            for b in range(4):
                tb = tw * 4 + b
                srcv = qT[p][hsl, tb * P:(tb + 1) * P]
                dstv = qTz[p][hsl, tb * 2 * P + h * P:tb * 2 * P + (h + 1) * P]
                if eng == 2:
                    nc.gpsimd.tensor_copy(out=dstv, in_=srcv)
                elif eng == 1:
                    nc.scalar.copy(out=dstv, in_=srcv)
                elif (h * 4 + b) % 2 == 0:
                    nc.vector.tensor_copy(out=dstv, in_=srcv)
                else:
                    nc.scalar.copy(out=dstv, in_=srcv)

    for p in range(NP):
        emit_norm_scale(p)
    for tb in range(8, NTB):
        emit_vproj(tb, tb % 2)
        drain_tr(2)
    drain_tr(99)
    # qTz windows 0-1 on DVE/scalar (consumed within ~10us of attention
    # start; GpSimd is too slow to deliver tw1 by block 4); windows 2-3
    # stream on GpSimd during early attention
    for p in range(NP):
        emit_qtz(p, 0, p % 2)
    for p in range(NP):
        emit_qtz(p, 1, 1)
    for tw in range(2, NTW):
        for p in range(NP):
            emit_qtz(p, tw, 2)
    phA.close()

    # =======================================================================
    # Attention: per token block, 3 pairs round-robin, two heads fused per
    # matmul (zero-interleaved qTz for scores, garbage-quadrant state,
    # block-diagonal S for inter).  15 matmuls per block instead of 24 --
    # short-N matmuls pay a ~150ns weight-load tax each.
    # =======================================================================
    phB = ExitStack()
    ps_wei = phB.enter_context(
        tc.tile_pool(name="ps_wei", bufs=2, space="PSUM"))
    ps_S = phB.enter_context(tc.tile_pool(name="ps_S", bufs=1, space="PSUM"))
    ps_out = phB.enter_context(
        tc.tile_pool(name="ps_out", bufs=3, space="PSUM"))

    # one matmul-write region per PSUM bank generation (hw rule)
    pS = [ps_S.tile([P, TW], F32, name=f"pS{p}", tag=f"S{p}")
          for p in range(NP)]
    po = [None] * NP           # per-pair [P, TW] psum accumulators
    wei_sb = [None] * NP       # current block's masked scores per pair
    S_bd = [None] * NP         # block-diag snapshot of S through prev block

    def emit_score(tb, p):
        # fused two-head scores: [s, tA | tB] in one [P, 256] matmul
        tsl = slice(tb * P, (tb + 1) * P)
        pw = ps_wei.tile([P, TW], F32, name="pw", tag="wei")[:, 0:2 * P]
        nc.tensor.matmul(pw, lhsT=kT[p][:, tsl],
                         rhs=qTz[p][:, tb * 2 * P:(tb + 1) * 2 * P],
                         start=True, stop=True)
        w = weipool.tile([P, 2 * P], dt, name=f"w{p}", tag=f"wsb{p}", bufs=3)
        nc.vector.tensor_tensor(out=w[:], in0=pw, in1=mask_sb[:, 0:2 * P],
                                op=OP.mult)
        wei_nxt[p] = w

    def emit_attn_tb(tb):
        tb4 = tb % 4
        tw = tb // 4
        tsl = slice(tb * P, (tb + 1) * P)
        osl = slice(tb4 * P, (tb4 + 1) * P)
        S_cur = list(S_bd)
        last = tb == NTB - 1

        if tb4 == 0:
            for p in range(NP):
                po[p] = ps_out.tile([P, TW], F32, name=f"po{p}", tag="po")

        # next block's scores for two pairs up front (their masked copies
        # run on DVE while this block's matmuls stream)
        if not last:
            emit_score(tb + 1, 0)
            emit_score(tb + 1, 1)
            # fused state: S[p] += k_blk^T v_blk, both heads + garbage
            # quadrants in one [128,128] matmul per pair
            for p in range(NP):
                nc.tensor.matmul(pS[p][:, 0:P],
                                 lhsT=kn_t[tb][:, p * P:(p + 1) * P],
                                 rhs=v_t[tb][:, p * P:(p + 1) * P],
                                 start=(tb == 0), stop=(tb == NTB - 2),
                                 skip_group_check=True)
            # block-diag snapshot: S_bd = pS * bdmask (zeroes the garbage);
            # p0 direct on DVE, p1/p2 bounced scalar -> gpsimd
            for p in range(NP):
                sb = sspool.tile([P, P], dt, name=f"Sbd{p}", tag=f"Sbd{p}")
                nc.vector.tensor_tensor(out=sb[:], in0=pS[p][:, 0:P],
                                        in1=bd_sb[:], op=OP.mult)
                S_bd[p] = sb
            emit_score(tb + 1, 2)

        # intra: per (pair, head); inter: fused per pair
        for p in range(NP):
            w = wei_sb[p]
            for h in range(2):
                hsl = slice(h * D, (h + 1) * D)
                vh = v_t[tb][:, p * P + h * D:p * P + (h + 1) * D]
                nc.tensor.matmul(po[p][hsl, osl], lhsT=vh,
                                 rhs=w[:, h * P:(h + 1) * P],
                                 start=True, stop=(tb == 0),
                                 skip_group_check=True)
            if tb > 0:
                nc.tensor.matmul(po[p][:, osl], lhsT=S_cur[p][:],
                                 rhs=qT[p][:, tsl], start=False, stop=True,
                                 skip_group_check=True)

        # close out the 512-wide output window
        if tb4 == 3:
            wsl = slice(tw * TW, (tw + 1) * TW)
            for p in range(NP):
                nc.scalar.copy(out=outT[p][:, wsl], in_=po[p])

    wei_nxt = [None] * NP
    for p in range(NP):
        emit_score(0, p)
    for tb in range(NTB):
        wei_sb = list(wei_nxt)
        emit_attn_tb(tb)
    phB.close()

    # =======================================================================
    # Output projection tail: out[t, :] = sum_p outT[p]^T @ wp[p]
    # =======================================================================
    ps_pr = ctx.enter_context(tc.tile_pool(name="ps_pr", bufs=6, space="PSUM"))
    for tb in range(NTB):
        tsl = slice(tb * P, (tb + 1) * P)
        pr = prpool.tile([P, C], dt, name="pr", tag="pr")
        for (n0, nsz) in ((0, TW), (TW, C - TW)):
            pp = ps_pr.tile([P, TW], F32, name="pp", tag="pr")[:, :nsz]
            for p in range(NP):
                nc.tensor.matmul(pp, lhsT=outT[p][:, tsl],
                                 rhs=wp[:, p * C + n0:p * C + n0 + nsz],
                                 start=(p == 0), stop=(p == NP - 1))
            (nc.scalar.copy if (tb + (n0 > 0)) % 2 else nc.vector.tensor_copy)(
                out=pr[:, n0:n0 + nsz], in_=pp)
        nc.sync.dma_start(out[tsl, :], pr[:])


# ---------------------------------------------------------------------------
# Host side: shard, run on 8 cores, unshard.
# ---------------------------------------------------------------------------

_DT_NAME = os.environ.get("KERNEL_DT", "bf16")
DT = {"f32": mybir.dt.float32, "bf16": mybir.dt.bfloat16}[_DT_NAME]
_NP_DT = {"f32": np.float32, "bf16": ml_dtypes.bfloat16}[_DT_NAME]

_CACHED = {}


def _get_nc():
    key = (DT,)
    if key not in _CACHED:
        _CACHED[key] = build_nc(DT)
    return _CACHED[key]


def _pack_rows(a, blk=P):
    """[N*blk, M] -> [blk, N*M] with row-block n at column block n."""
    n = a.shape[0] // blk
    return np.ascontiguousarray(
        a.reshape(n, blk, a.shape[1]).transpose(1, 0, 2).reshape(blk, -1))


def make_in_maps(x, Wq, Wk, Wv, Wp, bp):
    x = np.asarray(x, np.float32)
    Wq = np.asarray(Wq, np.float32)
    Wk = np.asarray(Wk, np.float32)
    Wv = np.asarray(Wv, np.float32)
    Wp = np.asarray(Wp, np.float32)

    cast = lambda a: np.ascontiguousarray(a).astype(_NP_DT)
    # mask[s, t] = 1 where t >= s (keep, causal incl. diagonal), x3 pairs
    mask1 = np.triu(np.ones((P, P), np.float32))
    mask3 = cast(np.concatenate([mask1] * NP, axis=1))
    ident = cast(np.eye(P, dtype=np.float32))
    bd = np.zeros((P, P), np.float32)
    bd[0:D, 0:D] = 1.0
    bd[D:P, D:P] = 1.0
    bdmask = cast(bd)
    HG = H // 2  # heads per group

    in_maps = []
    for core in range(8):
        b, g = divmod(core, 2)
        hsl = slice(g * HG, (g + 1) * HG)
        wq_s = Wq[hsl].transpose(1, 0, 2).reshape(C, HG * D)
        wk_s = Wk[hsl].transpose(1, 0, 2).reshape(C, HG * D)
        wv_s = Wv[hsl].transpose(1, 0, 2).reshape(C, HG * D)
        wp_s = Wp[g * HG * D:(g + 1) * HG * D, :]
        # x: [C, T] -> [128, tw-major (tw, kc, j)] packed
        xT_b = x[b].T.reshape(KC, P, NTW, TW)
        xp = xT_b.transpose(1, 2, 0, 3).reshape(P, KC * T)
        in_maps.append({
            "xt": cast(xp),
            "wv": cast(_pack_rows(wv_s)),
            "wq": cast(_pack_rows(wq_s)),
            "wk": cast(_pack_rows(wk_s)),
            "wp": cast(_pack_rows(wp_s)),
            "mask3": mask3,
            "ident": ident,
            "bdmask": bdmask,
        })
    return in_maps


def kernel(x, Wq, Wk, Wv, Wp, bp):
    from concourse.bass_utils import run_bass_kernel_spmd

    in_maps = make_in_maps(x, Wq, Wk, Wv, Wp, bp)
    nc = _get_nc()
    res = run_bass_kernel_spmd(nc, in_maps, core_ids=list(range(8)))
    parts = [np.asarray(r["out"], np.float32) for r in res.results]
    bp32 = np.asarray(bp, np.float32)
    return np.stack([parts[2 * b] + parts[2 * b + 1] + bp32 for b in range(B)])


# revision 7
# speedup vs baseline: 1.0401x; 1.0401x over previous
"""Trainium2 Bass kernel for nn_MultiHeadAttention_28819230556860 (v2).

SimA (softmax-free) multi-head attention in chunked linear form:
    out_blk = tril(q k^T) v_blk + S^T q_blk,   S += k_blk v_blk^T

v2 restructuring vs v1 (169us):
  * k is projected ONCE (head-major) and transposed on the PE array to get
    the token-major copy needed for the state update (saves a full C x C
    projection per core).
  * Inputs are host-packed into SBUF layout so each tensor is a single
    dma_start (the sync engine serializes dma dispatches at ~600ns each;
    50 dispatches starved phase A of x data).
  * All phases are emitted so the tensor engine never idles: v-projection
    interleaved with q/k stage-1, attention processes the 3 head-pairs
    round-robin with next-block scores pipelined inside the current block,
    output projection as a dense tail.  Continuous PE occupancy keeps the
    engine at the 2.4 GHz p-state instead of 1.2 GHz.
  * PSUM->SBUF copies spread across Vector/Scalar so neither gates the PE.

PSUM zero-region rule: start=True marks the touched partitions' ENTIRE 2KB
bank row pending-zero.  When several accumulation groups share a bank, only
the first matmul in the bank may use start=True; later groups' first writes
find pending-zero bytes and reset lazily.

Sharding: 8 cores = 4 batches x 2 head-groups (6 heads each).  Each core
computes a partial (T, C) output (row-sharded Wp); the host sums core pairs.
"""

import os
import sys

sys.path.insert(0, "/opt/trn_rl_repo")

from contextlib import ExitStack

import ml_dtypes
import numpy as np

import concourse.bass as bass
import concourse.tile as tile
from concourse import bacc, mybir

# ---------------------------------------------------------------------------
# Problem constants (hardcoded from the reference nn.Module).
# ---------------------------------------------------------------------------
B, T, C = 4, 2048, 768
H, D = 12, 64
P = 128
KC = C // P          # 6 contraction tiles over the embedding dim
NP = 3               # head-pairs per core (6 heads, 2 per 128 partitions)
NPP = NP * P         # 384
TW = 512             # t-window for 512-wide psum tiles
NTW = T // TW        # 4
NTB = T // P         # 16 token blocks
XW = KC * TW         # packed x columns per t-window (3072)
SCALE = float(C) ** -0.5
EPS = 1e-12

F32 = mybir.dt.float32


def build_nc(dt=F32):
    nc = bacc.Bacc("TRN2", target_bir_lowering=False, debug=False,
                   enable_asserts=False)

    # All inputs host-packed to [128, .] SBUF layout; single DMA each.
    xt_d = nc.dram_tensor("xt", [P, KC * T], dt, kind="ExternalInput").ap()
    wv_d = nc.dram_tensor("wv", [P, KC * NPP], dt, kind="ExternalInput").ap()
    wq_d = nc.dram_tensor("wq", [P, KC * NPP], dt, kind="ExternalInput").ap()
    wk_d = nc.dram_tensor("wk", [P, KC * NPP], dt, kind="ExternalInput").ap()
    wp_d = nc.dram_tensor("wp", [P, NP * C], dt, kind="ExternalInput").ap()
    mask_d = nc.dram_tensor("mask3", [P, NPP], dt, kind="ExternalInput").ap()
    id_d = nc.dram_tensor("ident", [P, P], dt, kind="ExternalInput").ap()
    bd_d = nc.dram_tensor("bdmask", [P, P], dt, kind="ExternalInput").ap()
    out = nc.dram_tensor("out", [T, C], dt, kind="ExternalOutput").ap()

    with tile.TileContext(nc) as tc:
        with ExitStack() as ctx:
            _body(ctx, tc, dt, xt_d, wv_d, wq_d, wk_d, wp_d, mask_d, id_d,
                  bd_d, out)
    nc.compile()
    return nc


def _body(ctx, tc, dt, xt_d, wv_d, wq_d, wk_d, wp_d, mask_d, id_d, bd_d,
          out):
    nc = tc.nc
    OP = mybir.AluOpType
    AF = mybir.ActivationFunctionType

    consts = ctx.enter_context(tc.tile_pool(name="consts", bufs=1))

    # ---- SBUF const tiles --------------------------------------------------
    xt = consts.tile([P, KC * T], dt, name="xt", tag="xt")
    wv = consts.tile([P, KC * NPP], dt, name="wv", tag="wv")
    wq = consts.tile([P, KC * NPP], dt, name="wq", tag="wq")
    wk = consts.tile([P, KC * NPP], dt, name="wk", tag="wk")
    wp = consts.tile([P, NP * C], dt, name="wp", tag="wp")
    mask_sb = consts.tile([P, NPP], dt, name="mask_sb", tag="mask_sb")
    id_sb = consts.tile([P, P], dt, name="id_sb", tag="id_sb")
    bd_sb = consts.tile([P, P], dt, name="bd_sb", tag="bd_sb")

    # packed x slices: window tw, contraction tile kc
    def xw(tw, kc, j0, n):
        c0 = tw * XW + kc * TW + j0
        return xt[:, c0:c0 + n]

    # ---- DMA schedule (few large transfers, prioritized) -------------------
    nc.sync.dma_start(wv[:], wv_d[:])
    nc.sync.dma_start(xt[:, 0:XW], xt_d[:, 0:XW])
    nc.sync.dma_start(wq[:], wq_d[:])
    nc.sync.dma_start(wk[:], wk_d[:])
    for tw in range(1, NTW):
        nc.sync.dma_start(xt[:, tw * XW:(tw + 1) * XW],
                          xt_d[:, tw * XW:(tw + 1) * XW])
    nc.sync.dma_start(mask_sb[:], mask_d[:])
    nc.sync.dma_start(id_sb[:], id_d[:])
    nc.sync.dma_start(bd_sb[:], bd_d[:])
    nc.sync.dma_start(wp[:], wp_d[:])

    # ---- SBUF pools --------------------------------------------------------
    vpool = ctx.enter_context(tc.tile_pool(name="vpool", bufs=1))
    qkpool = ctx.enter_context(tc.tile_pool(name="qkpool", bufs=1))
    spool = ctx.enter_context(tc.tile_pool(name="spool", bufs=2))
    opool = ctx.enter_context(tc.tile_pool(name="opool", bufs=1))
    weipool = ctx.enter_context(tc.tile_pool(name="weipool", bufs=4))
    sspool = ctx.enter_context(tc.tile_pool(name="sspool", bufs=4))
    prpool = ctx.enter_context(tc.tile_pool(name="prpool", bufs=4))

    v_t = [vpool.tile([P, NPP], dt, name=f"v{tb}", tag=f"v{tb}")
           for tb in range(NTB)]
    kn_t = [vpool.tile([P, NPP], dt, name=f"kn{tb}", tag=f"kn{tb}")
            for tb in range(NTB)]
    qT = [qkpool.tile([P, T], dt, name=f"qT{p}", tag=f"qT{p}")
          for p in range(NP)]
    kT = [qkpool.tile([P, T], dt, name=f"kT{p}", tag=f"kT{p}")
          for p in range(NP)]
    qTz = [qkpool.tile([P, 2 * T], dt, name=f"qTz{p}", tag=f"qTz{p}")
           for p in range(NP)]
    for p in range(NP):
        nc.gpsimd.memset(qTz[p][:], 0.0)
    outT = [opool.tile([P, T], dt, name=f"outT{p}", tag=f"outT{p}")
            for p in range(NP)]
    sqp = [spool.tile([P, NTW], F32, name=f"sqp{p}", tag=f"sqp{p}", bufs=1)
           for p in range(NP)]
    skp = [spool.tile([P, NTW], F32, name=f"skp{p}", tag=f"skp{p}", bufs=1)
           for p in range(NP)]
    fac = [spool.tile([P, 1], F32, name=f"fac{p}", tag=f"fac{p}", bufs=1)
           for p in range(NP)]

    # =======================================================================
    # Phase A: projections (v token-major; q,k head-major), k transposes,
    # norm factors.  Emitted so the PE stream has no dependency stalls.
    # =======================================================================
    phA = ExitStack()
    ps_mm = phA.enter_context(tc.tile_pool(name="ps_mm", bufs=3, space="PSUM"))
    ps_tr = phA.enter_context(tc.tile_pool(name="ps_tr", bufs=4, space="PSUM"))

    def emit_vproj(tb, eng):
        tw, j0 = tb // 4, (tb % 4) * P
        pv = ps_mm.tile([P, TW], F32, name="pv", tag="mm")[:, :NPP]
        for kc in range(KC):
            nc.tensor.matmul(pv, lhsT=xw(tw, kc, j0, P),
                             rhs=wv[:, kc * NPP:(kc + 1) * NPP],
                             start=(kc == 0), stop=(kc == KC - 1))
        (nc.vector.tensor_copy if eng == 0 else nc.scalar.copy)(
            out=v_t[tb][:], in_=pv)

    def emit_stage1(p, tw, which):
        wsl = slice(tw * TW, (tw + 1) * TW)
        w_t = wq if which == "q" else wk
        dst = qT[p] if which == "q" else kT[p]
        acc = sqp[p] if which == "q" else skp[p]
        pq = ps_mm.tile([P, TW], F32, name="pq", tag="mm")
        for kc in range(KC):
            nc.tensor.matmul(
                pq, lhsT=w_t[:, kc * NPP + p * P:kc * NPP + (p + 1) * P],
                rhs=xw(tw, kc, 0, TW),
                start=(kc == 0), stop=(kc == KC - 1))
        nc.vector.tensor_copy(out=dst[:, wsl], in_=pq)
        scr = spool.tile([P, TW], F32, name="scr", tag="scr", bufs=2)
        nc.scalar.activation(out=scr[:], in_=pq, func=AF.Square,
                             accum_out=acc[:, tw:tw + 1])

    pending_tr = []

    def drain_tr(n):
        # kT[p][:, block] -> token-major kn tiles, via PE transpose.  Emitted
        # a couple at a time between long matmul groups so the PE never
        # queues behind the psum->sbuf copies.
        for _ in range(min(n, len(pending_tr))):
            p, tb = pending_tr.pop(0)
            tsl = slice(tb * P, (tb + 1) * P)
            pt = ps_tr.tile([P, P], dt, name="pt", tag="tr")
            nc.tensor.transpose(pt, kT[p][:, tsl], id_sb[:])
            ((nc.vector.tensor_copy if (p + tb) % 2 else nc.scalar.copy)
             (out=kn_t[tb][:, p * P:(p + 1) * P], in_=pt))

    def emit_norm_scale(p):
        # fac[d] = SCALE / max(sqrt(||q_d||^2 * ||k_d||^2), eps); fold into q
        nq = spool.tile([P, 1], F32, name="nq", tag="nq")
        nk = spool.tile([P, 1], F32, name="nk", tag="nk")
        s4 = spool.tile([P, NTW], F32, name="s4", tag="s4")
        nc.scalar.activation(out=s4[:], in_=sqp[p][:], func=AF.Copy,
                             accum_out=nq[:])
        s5 = spool.tile([P, NTW], F32, name="s5", tag="s4")
        nc.scalar.activation(out=s5[:], in_=skp[p][:], func=AF.Copy,
                             accum_out=nk[:])
        nc.vector.tensor_mul(fac[p][:], nq[:], nk[:])
        nc.scalar.sqrt(fac[p][:], fac[p][:])
        nc.vector.tensor_scalar_max(fac[p][:], fac[p][:], EPS)
        nc.vector.reciprocal(fac[p][:], fac[p][:])
        nc.vector.tensor_scalar_mul(fac[p][:], fac[p][:], SCALE)
        for tw in range(NTW):
            wsl = slice(tw * TW, (tw + 1) * TW)
            if tw % 2:
                nc.scalar.mul(out=qT[p][:, wsl], in_=qT[p][:, wsl],
                              mul=fac[p][:])
            else:
                nc.vector.tensor_scalar_mul(qT[p][:, wsl], qT[p][:, wsl],
                                            fac[p][:])

    # --- phase A emission order (PE stream) ---
    for tw in range(NTW):
        emit_vproj(2 * tw, 0)
        drain_tr(2)
        emit_vproj(2 * tw + 1, 1)
        drain_tr(2)
        for p in range(NP):
            emit_stage1(p, tw, "q")
            drain_tr(2)
        for p in range(NP):
            emit_stage1(p, tw, "k")
            drain_tr(2)
            pending_tr.extend((p, tw * 4 + i) for i in range(4))
    def emit_qtz(p, tw, eng):
        # zero-interleaved q for fused two-head scores (zeros laid down by
        # the one-time memset above):
        # qTz[0:64, tb*256+0:128] = qA(tb), qTz[64:128, tb*256+128:256] = qB(tb)
        for h in range(2):
            hsl = slice(h * D, (h + 1) * D)
            for b in range(4):
                tb = tw * 4 + b
                srcv = qT[p][hsl, tb * P:(tb + 1) * P]
                dstv = qTz[p][hsl, tb * 2 * P + h * P:tb * 2 * P + (h + 1) * P]
                if eng == 2:
                    nc.gpsimd.tensor_copy(out=dstv, in_=srcv)
                elif eng == 1:
                    nc.scalar.copy(out=dstv, in_=srcv)
                elif (h * 4 + b) % 2 == 0:
                    nc.vector.tensor_copy(out=dstv, in_=srcv)
                else:
                    nc.scalar.copy(out=dstv, in_=srcv)

    for p in range(NP):
        emit_norm_scale(p)
    for tb in range(8, NTB):
        emit_vproj(tb, tb % 2)
        drain_tr(2)
    drain_tr(99)
    # qTz windows 0-1 on DVE/scalar (consumed within ~10us of attention
    # start; GpSimd is too slow to deliver tw1 by block 4); windows 2-3
    # stream on GpSimd during early attention
    for p in range(NP):
        emit_qtz(p, 0, p % 2)
    for p in range(NP):
        emit_qtz(p, 1, 1)
    for tw in range(2, NTW):
        for p in range(NP):
            emit_qtz(p, tw, 2)
    phA.close()

    # =======================================================================
    # Attention: per token block, 3 pairs round-robin, two heads fused per
    # matmul (zero-interleaved qTz for scores, garbage-quadrant state,
    # block-diagonal S for inter).  15 matmuls per block instead of 24 --
    # short-N matmuls pay a ~150ns weight-load tax each.
    # =======================================================================
    phB = ExitStack()
    ps_wei = phB.enter_context(
        tc.tile_pool(name="ps_wei", bufs=2, space="PSUM"))
    ps_S = phB.enter_context(tc.tile_pool(name="ps_S", bufs=1, space="PSUM"))
    ps_out = phB.enter_context(
        tc.tile_pool(name="ps_out", bufs=3, space="PSUM"))

    # one matmul-write region per PSUM bank generation (hw rule)
    pS = [ps_S.tile([P, TW], F32, name=f"pS{p}", tag=f"S{p}")
          for p in range(NP)]
    po = [None] * NP           # per-pair [P, TW] psum accumulators
    wei_sb = [None] * NP       # current block's masked scores per pair
    S_bd = [None] * NP         # block-diag snapshot of S through prev block

    def emit_score(tb, p):
        # fused two-head scores: [s, tA | tB] in one [P, 256] matmul
        tsl = slice(tb * P, (tb + 1) * P)
        pw = ps_wei.tile([P, TW], F32, name="pw", tag="wei")[:, 0:2 * P]
        nc.tensor.matmul(pw, lhsT=kT[p][:, tsl],
                         rhs=qTz[p][:, tb * 2 * P:(tb + 1) * 2 * P],
                         start=True, stop=True)
        w = weipool.tile([P, 2 * P], dt, name=f"w{p}", tag=f"wsb{p}", bufs=3)
        nc.vector.tensor_tensor(out=w[:], in0=pw, in1=mask_sb[:, 0:2 * P],
                                op=OP.mult)
        wei_nxt[p] = w

    def emit_attn_tb(tb):
        tb4 = tb % 4
        tw = tb // 4
        tsl = slice(tb * P, (tb + 1) * P)
        osl = slice(tb4 * P, (tb4 + 1) * P)
        S_cur = list(S_bd)
        last = tb == NTB - 1

        if tb4 == 0:
            for p in range(NP):
                po[p] = ps_out.tile([P, TW], F32, name=f"po{p}", tag="po")

        # next block's scores for two pairs up front (their masked copies
        # run on DVE while this block's matmuls stream)
        if not last:
            emit_score(tb + 1, 0)
            emit_score(tb + 1, 1)
            # fused state: S[p] += k_blk^T v_blk, both heads + garbage
            # quadrants in one [128,128] matmul per pair
            for p in range(NP):
                nc.tensor.matmul(pS[p][:, 0:P],
                                 lhsT=kn_t[tb][:, p * P:(p + 1) * P],
                                 rhs=v_t[tb][:, p * P:(p + 1) * P],
                                 start=(tb == 0), stop=(tb == NTB - 2),
                                 skip_group_check=True)
            # block-diag snapshot: S_bd = pS * bdmask (zeroes the garbage);
            # p0 direct on DVE, p1/p2 bounced scalar -> gpsimd
            for p in range(NP):
                sb = sspool.tile([P, P], dt, name=f"Sbd{p}", tag=f"Sbd{p}")
                nc.vector.tensor_tensor(out=sb[:], in0=pS[p][:, 0:P],
                                        in1=bd_sb[:], op=OP.mult)
                S_bd[p] = sb
            emit_score(tb + 1, 2)

        # intra: per (pair, head); inter: fused per pair
        for p in range(NP):
            w = wei_sb[p]
            for h in range(2):
                hsl = slice(h * D, (h + 1) * D)
                vh = v_t[tb][:, p * P + h * D:p * P + (h + 1) * D]
                nc.tensor.matmul(po[p][hsl, osl], lhsT=vh,
                                 rhs=w[:, h * P:(h + 1) * P],
                                 start=True, stop=(tb == 0),
                                 skip_group_check=True)
            if tb > 0:
                nc.tensor.matmul(po[p][:, osl], lhsT=S_cur[p][:],
                                 rhs=qT[p][:, tsl], start=False, stop=True,
                                 skip_group_check=True)

        # close out the 512-wide output window
        if tb4 == 3:
            wsl = slice(tw * TW, (tw + 1) * TW)
            for p in range(NP):
                nc.scalar.copy(out=outT[p][:, wsl], in_=po[p])

    wei_nxt = [None] * NP
    for p in range(NP):
        emit_score(0, p)
    for tb in range(NTB):
        wei_sb = list(wei_nxt)
        emit_attn_tb(tb)
    phB.close()

    # =======================================================================
    # Output projection tail: out[t, :] = sum_p outT[p]^T @ wp[p]
    # =======================================================================
    ps_pr = ctx.enter_context(tc.tile_pool(name="ps_pr", bufs=6, space="PSUM"))
    for tb in range(NTB):
        tsl = slice(tb * P, (tb + 1) * P)
        pr = prpool.tile([P, C], dt, name="pr", tag="pr")
        for (n0, nsz) in ((0, TW), (TW, C - TW)):
            pp = ps_pr.tile([P, TW], F32, name="pp", tag="pr")[:, :nsz]
            for p in range(NP):
                nc.tensor.matmul(pp, lhsT=outT[p][:, tsl],
                                 rhs=wp[:, p * C + n0:p * C + n0 + nsz],
                                 start=(p == 0), stop=(p == NP - 1))
            (nc.scalar.copy if (tb + (n0 > 0)) % 2 else nc.vector.tensor_copy)(
                out=pr[:, n0:n0 + nsz], in_=pp)
        nc.sync.dma_start(out[tsl, :], pr[:])


# ---------------------------------------------------------------------------
# Host side: shard, run on 8 cores, unshard.
# ---------------------------------------------------------------------------

_DT_NAME = os.environ.get("KERNEL_DT", "bf16")
DT = {"f32": mybir.dt.float32, "bf16": mybir.dt.bfloat16}[_DT_NAME]
_NP_DT = {"f32": np.float32, "bf16": ml_dtypes.bfloat16}[_DT_NAME]

_CACHED = {}


def _get_nc():
    key = (DT,)
    if key not in _CACHED:
        _CACHED[key] = build_nc(DT)
    return _CACHED[key]


def _pack_rows(a, blk=P):
    """[N*blk, M] -> [blk, N*M] with row-block n at column block n."""
    n = a.shape[0] // blk
    return np.ascontiguousarray(
        a.reshape(n, blk, a.shape[1]).transpose(1, 0, 2).reshape(blk, -1))


def make_in_maps(x, Wq, Wk, Wv, Wp, bp):
    # cast to bf16 FIRST: halves the memory traffic of the packing
    # transposes and commutes with them bit-for-bit
    x = np.asarray(x, np.float32).astype(_NP_DT)
    Wq = np.asarray(Wq, np.float32).astype(_NP_DT)
    Wk = np.asarray(Wk, np.float32).astype(_NP_DT)
    Wv = np.asarray(Wv, np.float32).astype(_NP_DT)
    Wp = np.asarray(Wp, np.float32).astype(_NP_DT)

    cast = lambda a: np.ascontiguousarray(a).astype(_NP_DT)
    # mask[s, t] = 1 where t >= s (keep, causal incl. diagonal), x3 pairs
    mask1 = np.triu(np.ones((P, P), np.float32))
    mask3 = cast(np.concatenate([mask1] * NP, axis=1))
    ident = cast(np.eye(P, dtype=np.float32))
    bd = np.zeros((P, P), np.float32)
    bd[0:D, 0:D] = 1.0
    bd[D:P, D:P] = 1.0
    bdmask = cast(bd)
    HG = H // 2  # heads per group

    # per-group tensors (2 groups, not 8 cores)
    grp = []
    for g in range(2):
        hsl = slice(g * HG, (g + 1) * HG)
        wq_s = Wq[hsl].transpose(1, 0, 2).reshape(C, HG * D)
        wk_s = Wk[hsl].transpose(1, 0, 2).reshape(C, HG * D)
        wv_s = Wv[hsl].transpose(1, 0, 2).reshape(C, HG * D)
        wp_s = Wp[g * HG * D:(g + 1) * HG * D, :]
        grp.append({
            "wv": cast(_pack_rows(wv_s)),
            "wq": cast(_pack_rows(wq_s)),
            "wk": cast(_pack_rows(wk_s)),
            "wp": cast(_pack_rows(wp_s)),
        })
    # per-batch packed x: [C, T] -> [128, tw-major (tw, kc, j)]
    xb = []
    for b in range(B):
        xT_b = x[b].T.reshape(KC, P, NTW, TW)
        xb.append(cast(xT_b.transpose(1, 2, 0, 3).reshape(P, KC * T)))

    in_maps = []
    for core in range(8):
        b, g = divmod(core, 2)
        in_maps.append({
            "xt": xb[b],
            **grp[g],
            "mask3": mask3,
            "ident": ident,
            "bdmask": bdmask,
        })
    return in_maps


def kernel(x, Wq, Wk, Wv, Wp, bp):
    from concourse.bass_utils import run_bass_kernel_spmd

    in_maps = make_in_maps(x, Wq, Wk, Wv, Wp, bp)
    nc = _get_nc()
    res = run_bass_kernel_spmd(nc, in_maps, core_ids=list(range(8)))
    parts = [np.asarray(r["out"], np.float32) for r in res.results]
    bp32 = np.asarray(bp, np.float32)
    return np.stack([parts[2 * b] + parts[2 * b + 1] + bp32 for b in range(B)])
